# revision 6
# baseline (speedup 1.0000x reference)
"""CenterLoss kernel for Trainium2 (8 NeuronCores, SPMD data-parallel).

Reference semantics: loss = clip(distmat * onehot(labels), 1e-12, 1e12).sum() / B
where distmat[i,j] = ||x_i - c_j||^2.  The mask keeps only distmat[i, labels_i];
the other B*(C-1) masked-out zeros clip to 1e-12, contributing an exact
constant B*(C-1)*1e-12 that we add on the host.  So the device computes
sum_i ||x_i - centers[labels_i]||^2 with a gather instead of the O(B*C*D)
distance matrix.

Device work per core (512 of the 4096 batch rows), per 128-row tile:
  - DMA the x tile [128, 512] into SBUF
  - indirect-DMA gather of centers[labels] rows (one row per partition)
  - fused:  the gather applies CCE add into the x tile, which the host
    pre-negated, giving d = c - x in the DMA engine (zero DVE work)
    plain:  DVE subtract d = x - c
  - ACT Square + free-axis accumulate -> per-partition partial sums
  - DMA out the [128, 4] partial-sum tile
Host: sum the 8 x [128, 4] partials in f64, add the clip constant, divide by B.
"""

import numpy as np

BATCH = 4096
FEAT = 512
NUM_CLASSES = 10000
N_CORES = 8
ROWS = BATCH // N_CORES  # 512 rows per core
P = 128
NT = ROWS // P  # 4 tiles of 128 rows

FUSED = True  # gather-with-CCE-add variant (host passes -x)

_CACHE = {}


def _build_nc():
    import concourse.bacc as bacc
    import concourse.bass as bass
    import concourse.mybir as mybir
    from concourse.tile import TileContext

    nc = bacc.Bacc("TRN2", target_bir_lowering=False, debug=False)

    x = nc.dram_tensor("x", [ROWS, FEAT], mybir.dt.float32, kind="ExternalInput")
    # host pre-arranges labels so lab[p, n] = labels[n*128 + p]
    labels = nc.dram_tensor("labels", [P, NT], mybir.dt.int32, kind="ExternalInput")
    centers = nc.dram_tensor(
        "centers", [NUM_CLASSES, FEAT], mybir.dt.float32, kind="ExternalInput"
    )
    partial = nc.dram_tensor("partial", [P, NT], mybir.dt.float32, kind="ExternalOutput")

    with TileContext(nc) as tc:
        with (
            tc.tile_pool(name="io", bufs=1) as io,
            tc.tile_pool(name="work", bufs=NT) as work,
        ):
            lab_sb = io.tile([P, NT], mybir.dt.int32)
            nc.sync.dma_start(out=lab_sb[:], in_=labels.ap())
            acc = io.tile([P, NT], mybir.dt.float32)
            for t in range(NT):
                if FUSED:
                    d_sb = work.tile([P, FEAT], mybir.dt.float32, tag="d")
                    nc.sync.dma_start(out=d_sb[:], in_=x.ap()[t * P : (t + 1) * P, :])
                    nc.gpsimd.indirect_dma_start(
                        out=d_sb[:],
                        out_offset=None,
                        in_=centers.ap(),
                        in_offset=bass.IndirectOffsetOnAxis(
                            ap=lab_sb[:, t : t + 1], axis=0
                        ),
                        compute_op=mybir.AluOpType.add,
                    )
                else:
                    x_sb = work.tile([P, FEAT], mybir.dt.float32, tag="x")
                    nc.sync.dma_start(out=x_sb[:], in_=x.ap()[t * P : (t + 1) * P, :])
                    c_sb = work.tile([P, FEAT], mybir.dt.float32, tag="c")
                    nc.gpsimd.indirect_dma_start(
                        out=c_sb[:],
                        out_offset=None,
                        in_=centers.ap(),
                        in_offset=bass.IndirectOffsetOnAxis(
                            ap=lab_sb[:, t : t + 1], axis=0
                        ),
                    )
                    d_sb = work.tile([P, FEAT], mybir.dt.float32, tag="d")
                    nc.vector.tensor_tensor(
                        out=d_sb[:],
                        in0=x_sb[:],
                        in1=c_sb[:],
                        op=mybir.AluOpType.subtract,
                    )
                s_sb = work.tile([P, FEAT], mybir.dt.float32, tag="s")
                nc.scalar.activation(
                    out=s_sb[:],
                    in_=d_sb[:],
                    func=mybir.ActivationFunctionType.Square,
                    accum_out=acc[:, t : t + 1],
                )
            nc.sync.dma_start(out=partial.ap(), in_=acc[:])

    nc.compile()
    return nc


def _prepare_in_maps(x, centers, labels):
    x = np.asarray(x, dtype=np.float32)
    if FUSED:
        x = -x  # gather CCE computes c + (-x); squared, so sign is irrelevant
    x = np.ascontiguousarray(x)
    centers = np.ascontiguousarray(np.asarray(centers, dtype=np.float32))
    # lab[i][p, n] = labels[i*ROWS + n*P + p]
    lab = np.asarray(labels).astype(np.int32).reshape(N_CORES, NT, P)
    lab = np.ascontiguousarray(lab.transpose(0, 2, 1))
    xs = x.reshape(N_CORES, ROWS, FEAT)
    return [
        {"x": xs[i], "labels": lab[i], "centers": centers} for i in range(N_CORES)
    ]


def kernel(x, centers, labels):
    from concourse.bass_utils import run_bass_kernel_spmd

    if "nc" not in _CACHE:
        _CACHE["nc"] = _build_nc()
    nc = _CACHE["nc"]

    in_maps = _prepare_in_maps(x, centers, labels)
    res = run_bass_kernel_spmd(nc, in_maps, core_ids=list(range(N_CORES)))

    total = np.float64(0.0)
    for r in res.results:
        total += r["partial"].astype(np.float64).sum()
    total += np.float64(BATCH) * (NUM_CLASSES - 1) * 1e-12  # clip floor of masked zeros
    return np.float32(total / BATCH)


# revision 7
# speedup vs baseline: 1.2076x; 1.2076x over previous
"""Raw-bacc (no TileContext) CenterLoss kernel — hand-written semaphores.

Engine programs per core:
  sync   : DMA labels, DMA 4 x-tiles (one HWDGE ring, FIFO), final out DMA
  gpsimd : 4 indirect gathers with CCE add into the x tiles (SWDGE q0, FIFO)
  scalar : 4 Square+accum activations
Cumulative waits are sound because each DMA ring completes in FIFO order.
"""

import numpy as np

BATCH = 4096
FEAT = 512
NUM_CLASSES = 10000
N_CORES = 8
ROWS = BATCH // N_CORES
P = 128
NT = ROWS // P

_CACHE = {}


def _build_nc():
    import concourse.bacc as bacc
    import concourse.bass as bass
    import concourse.mybir as mybir

    # shrink the kernel semaphore range: the NEFF entry emits a per-engine
    # EVENT_SEMAPHORE_RANGE_CLEAR over this whole range (~31ns/sem) and the
    # exit resets it again; we use ~20 sems, not 106
    _orig_range = bass.get_kernel_semaphore_range
    bass.get_kernel_semaphore_range = lambda: range(150, 182)
    try:
        nc = bacc.Bacc("TRN2", target_bir_lowering=False, debug=False)
    finally:
        bass.get_kernel_semaphore_range = _orig_range

    x = nc.dram_tensor("x", [ROWS, FEAT], mybir.dt.float32, kind="ExternalInput")
    labels = nc.dram_tensor("labels", [P, NT], mybir.dt.int32, kind="ExternalInput")
    centers = nc.dram_tensor(
        "centers", [NUM_CLASSES, FEAT], mybir.dt.float32, kind="ExternalInput"
    )
    partial = nc.dram_tensor("partial", [P, NT], mybir.dt.float32, kind="ExternalOutput")

    with (
        nc.sbuf_tensor([P, NT, FEAT], mybir.dt.float32) as dbuf,
        nc.sbuf_tensor([P, NT], mybir.dt.int32) as lab_sb,
        nc.sbuf_tensor([P, NT, FEAT], mybir.dt.float32) as sq,
        nc.sbuf_tensor([P, NT], mybir.dt.float32) as acc,
        nc.sbuf_tensor([P, 1, FEAT], mybir.dt.float32) as warm,
        nc.sbuf_tensor([P, 1], mybir.dt.int32) as lab_dummy,
        nc.semaphore() as lsem,
        nc.semaphore() as x0sem,
        nc.semaphore() as x1sem,
        nc.semaphore() as x2sem,
        nc.semaphore() as x3sem,
        nc.semaphore() as osem,
        nc.semaphore() as g0sem,
        nc.semaphore() as g1sem,
        nc.semaphore() as g2sem,
        nc.semaphore() as g3sem,
        nc.semaphore() as asem,
        nc.semaphore() as msem,
        nc.semaphore() as wsem,
        nc.Block() as block,
    ):
        xsems = [x0sem, x1sem, x2sem, x3sem]
        gsems = [g0sem, g1sem, g2sem, g3sem]

        @block.sync
        def _(sync):
            sync.dma_start(
                out=dbuf[:, 0, :], in_=x.ap()[0:P, :]
            ).then_inc(xsems[0], 16)
            sync.wait_ge(asem, NT)
            sync.dma_start(out=partial.ap(), in_=acc[:]).then_inc(osem, 16)
            sync.wait_ge(osem, 16)

        @block.gpsimd
        def _(g):
            # warm up the SWDGE worker before inputs land: lazy first-use
            # init (~2.4us) then overlaps the entry phase
            g.memset(lab_dummy[:], 0).then_inc(msem, 1)
            g.wait_ge(msem, 1)
            g.indirect_dma_start(
                out=warm[:16, 0, :64],
                out_offset=None,
                in_=centers.ap(),
                in_offset=bass.IndirectOffsetOnAxis(ap=lab_dummy[:16, 0:1], axis=0),
            ).then_inc(wsem, 16)
            g.wait_ge(lsem, 16)
            for t in range(NT):
                g.wait_ge(xsems[t], 16)
                g.indirect_dma_start(
                    out=dbuf[:, t, :],
                    out_offset=None,
                    in_=centers.ap(),
                    in_offset=bass.IndirectOffsetOnAxis(
                        ap=lab_sb[:, t : t + 1], axis=0
                    ),
                    compute_op=mybir.AluOpType.add,
                ).then_inc(gsems[t], 16)

        @block.scalar
        def _(s):
            s.dma_start(out=lab_sb[:], in_=labels.ap()).then_inc(lsem, 16)
            for t in range(1, NT):
                s.dma_start(
                    out=dbuf[:, t, :], in_=x.ap()[t * P : (t + 1) * P, :]
                ).then_inc(xsems[t], 16)
            for t in range(NT):
                s.wait_ge(gsems[t], 16)
                s.activation(
                    out=sq[:, t, :],
                    in_=dbuf[:, t, :],
                    func=mybir.ActivationFunctionType.Square,
                    accum_out=acc[:, t : t + 1],
                ).then_inc(asem, 1)

    nc.compile()
    return nc


def _prepare_in_maps(x, centers, labels):
    x = -np.asarray(x, dtype=np.float32)  # CCE add computes c + (-x)
    x = np.ascontiguousarray(x)
    centers = np.ascontiguousarray(np.asarray(centers, dtype=np.float32))
    lab = np.asarray(labels).astype(np.int32).reshape(N_CORES, NT, P)
    lab = np.ascontiguousarray(lab.transpose(0, 2, 1))
    xs = x.reshape(N_CORES, ROWS, FEAT)
    return [
        {"x": xs[i], "labels": lab[i], "centers": centers} for i in range(N_CORES)
    ]


def kernel(x, centers, labels):
    from concourse.bass_utils import run_bass_kernel_spmd

    if "nc" not in _CACHE:
        _CACHE["nc"] = _build_nc()
    nc = _CACHE["nc"]

    in_maps = _prepare_in_maps(x, centers, labels)
    res = run_bass_kernel_spmd(nc, in_maps, core_ids=list(range(N_CORES)))

    total = np.float64(0.0)
    for r in res.results:
        total += r["partial"].astype(np.float64).sum()
    total += np.float64(BATCH) * (NUM_CLASSES - 1) * 1e-12
    return np.float32(total / BATCH)


# revision 8
# speedup vs baseline: 1.2382x; 1.0254x over previous
"""CenterLoss kernel for Trainium2 (8 NeuronCores, SPMD data-parallel).

Reference semantics: loss = clip(distmat * onehot(labels), 1e-12, 1e12).sum()/B
with distmat[i,j] = ||x_i - c_j||^2.  The one-hot mask keeps only
distmat[i, labels_i]; the B*(C-1) masked-out zeros clip to 1e-12, an exact
constant added on the host.  So each core gathers centers[labels] for its 512
batch rows and computes sum ||x_i - c_i||^2 — O(B*D) work instead of the
O(B*C*D) distance matrix.

Raw bacc (no TileContext), hand-written semaphores.  Per core:
  sync   : x0 tile DMA alone on the SP HWDGE ring (so it completes early and
           un-gates the first gather), final out DMA of the [128, 4]
           per-partition partial sums
  scalar : labels DMA + x1..x3 on the ACT HWDGE ring; 4x Square activation
           with free-axis accumulate
  gpsimd : tiny warmup indirect gather at boot (absorbs the SWDGE worker's
           lazy first-use init during the fixed NEFF entry phase), then 4x
           indirect row-gather from centers with CCE add fused into the
           host-negated x tiles: dbuf = c + (-x), squared so the sign cancels
  vector : idle (the subtract is fused into the gather DMA)
Host: pack labels as int32 [128, 4] per core (lab[p, t] = labels[t*128+p]),
negate x, sum the 8 partial tiles in f64, add the clip-floor constant,
divide by B.
"""

import numpy as np

BATCH = 4096
FEAT = 512
NUM_CLASSES = 10000
N_CORES = 8
ROWS = BATCH // N_CORES
P = 128
NT = ROWS // P

_CACHE = {}


def _build_nc():
    import concourse.bacc as bacc
    import concourse.bass as bass
    import concourse.mybir as mybir

    # shrink the kernel semaphore range: the NEFF entry emits a per-engine
    # EVENT_SEMAPHORE_RANGE_CLEAR over this whole range (~31ns/sem) and the
    # exit resets it again; we use ~20 sems, not 106
    _orig_range = bass.get_kernel_semaphore_range
    bass.get_kernel_semaphore_range = lambda: range(150, 182)
    try:
        nc = bacc.Bacc("TRN2", target_bir_lowering=False, debug=False)
    finally:
        bass.get_kernel_semaphore_range = _orig_range

    x = nc.dram_tensor("x", [ROWS, FEAT], mybir.dt.float32, kind="ExternalInput")
    labels = nc.dram_tensor("labels", [P, NT], mybir.dt.int32, kind="ExternalInput")
    centers = nc.dram_tensor(
        "centers", [NUM_CLASSES, FEAT], mybir.dt.float32, kind="ExternalInput"
    )
    partial = nc.dram_tensor("partial", [P, NT], mybir.dt.float32, kind="ExternalOutput")

    with (
        nc.sbuf_tensor([P, NT, FEAT], mybir.dt.float32) as dbuf,
        nc.sbuf_tensor([P, NT], mybir.dt.int32) as lab_sb,
        nc.sbuf_tensor([P, NT, FEAT], mybir.dt.float32) as sq,
        nc.sbuf_tensor([P, NT], mybir.dt.float32) as acc,
        nc.sbuf_tensor([P, 1, FEAT], mybir.dt.float32) as warm,
        nc.sbuf_tensor([P, 1], mybir.dt.int32) as lab_dummy,
        nc.semaphore() as lsem,
        nc.semaphore() as x0sem,
        nc.semaphore() as x1sem,
        nc.semaphore() as x2sem,
        nc.semaphore() as x3sem,
        nc.semaphore() as osem,
        nc.semaphore() as g0sem,
        nc.semaphore() as g1sem,
        nc.semaphore() as g2sem,
        nc.semaphore() as g3sem,
        nc.semaphore() as asem,
        nc.semaphore() as msem,
        nc.semaphore() as wsem,
        nc.Block() as block,
    ):
        xsems = [x0sem, x1sem, x2sem, x3sem]
        gsems = [g0sem, g1sem, g2sem, g3sem]

        @block.sync
        def _(sync):
            sync.dma_start(
                out=dbuf[:, 0, :], in_=x.ap()[0:P, :]
            ).then_inc(xsems[0], 16)
            sync.wait_ge(asem, NT)
            sync.dma_start(out=partial.ap(), in_=acc[:]).then_inc(osem, 16)
            sync.wait_ge(osem, 16)

        @block.gpsimd
        def _(g):
            # warm up the SWDGE worker before inputs land: lazy first-use
            # init (~2.4us) then overlaps the entry phase
            g.memset(lab_dummy[:], 0).then_inc(msem, 1)
            g.wait_ge(msem, 1)
            g.indirect_dma_start(
                out=warm[:16, 0, :64],
                out_offset=None,
                in_=centers.ap(),
                in_offset=bass.IndirectOffsetOnAxis(ap=lab_dummy[:16, 0:1], axis=0),
            ).then_inc(wsem, 16)
            g.wait_ge(lsem, 16)
            for t in range(NT):
                g.wait_ge(xsems[t], 16)
                g.indirect_dma_start(
                    out=dbuf[:, t, :],
                    out_offset=None,
                    in_=centers.ap(),
                    in_offset=bass.IndirectOffsetOnAxis(
                        ap=lab_sb[:, t : t + 1], axis=0
                    ),
                    compute_op=mybir.AluOpType.add,
                ).then_inc(gsems[t], 16)

        @block.scalar
        def _(s):
            s.dma_start(out=lab_sb[:], in_=labels.ap()).then_inc(lsem, 16)
            for t in range(1, NT):
                s.dma_start(
                    out=dbuf[:, t, :], in_=x.ap()[t * P : (t + 1) * P, :]
                ).then_inc(xsems[t], 16)
            for t in range(NT):
                s.wait_ge(gsems[t], 16)
                s.activation(
                    out=sq[:, t, :],
                    in_=dbuf[:, t, :],
                    func=mybir.ActivationFunctionType.Square,
                    accum_out=acc[:, t : t + 1],
                ).then_inc(asem, 1)

    nc.compile()
    return nc


def _prepare_in_maps(x, centers, labels):
    x = -np.asarray(x, dtype=np.float32)  # CCE add computes c + (-x)
    x = np.ascontiguousarray(x)
    centers = np.ascontiguousarray(np.asarray(centers, dtype=np.float32))
    lab = np.asarray(labels).astype(np.int32).reshape(N_CORES, NT, P)
    lab = np.ascontiguousarray(lab.transpose(0, 2, 1))
    xs = x.reshape(N_CORES, ROWS, FEAT)
    return [
        {"x": xs[i], "labels": lab[i], "centers": centers} for i in range(N_CORES)
    ]


def kernel(x, centers, labels):
    from concourse.bass_utils import run_bass_kernel_spmd

    if "nc" not in _CACHE:
        _CACHE["nc"] = _build_nc()
    nc = _CACHE["nc"]

    in_maps = _prepare_in_maps(x, centers, labels)
    res = run_bass_kernel_spmd(nc, in_maps, core_ids=list(range(N_CORES)))

    total = np.float64(0.0)
    for r in res.results:
        total += r["partial"].astype(np.float64).sum()
    total += np.float64(BATCH) * (NUM_CLASSES - 1) * 1e-12
    return np.float32(total / BATCH)


# revision 9
# speedup vs baseline: 1.3912x; 1.1236x over previous
"""CenterLoss kernel for Trainium2 (8 NeuronCores, SPMD data-parallel).

Reference semantics: loss = clip(distmat * onehot(labels), 1e-12, 1e12).sum()/B
with distmat[i,j] = ||x_i - c_j||^2.  The one-hot mask keeps only
distmat[i, labels_i]; the B*(C-1) masked-out zeros clip to 1e-12, an exact
constant added on the host.  So each core gathers centers[labels] for its 512
batch rows and computes sum ||x_i - c_i||^2 -- O(B*D) work instead of the
O(B*C*D) distance matrix.

Raw bacc (no TileContext), hand-written semaphores.  Per core:
  sync   : labels DMA first (2KB, sole gather dependency -> completes ~8.6us),
           then the x0 tile; final out DMA of the [128,4] partial sums
  scalar : x1..x3 tile DMAs on the ACT HWDGE ring; 4x Square activation with
           free-axis accumulate into acc
  gpsimd : tiny warmup indirect gather at boot (absorbs the SWDGE worker's
           lazy first-use init inside the fixed NEFF entry phase), then 4x
           128-row indirect gathers from centers -- unfused (plain writes):
           desc-gen is ~25% faster than CCE and transfers pace better, and
           the gathers need only the labels, so they never wait on x
  vector : 4x subtract dbuf = x - c, one per gathered tile
Host: pack labels as int32 [128, 4] per core (lab[p, t] = labels[t*128+p]),
sum the 8 partial tiles in f64, add the clip-floor constant, divide by B.
"""

import numpy as np

BATCH = 4096
FEAT = 512
NUM_CLASSES = 10000
N_CORES = 8
ROWS = BATCH // N_CORES
P = 128
NT = ROWS // P

_CACHE = {}


def _build_nc():
    import concourse.bacc as bacc
    import concourse.bass as bass
    import concourse.mybir as mybir

    # shrink the kernel semaphore range: the NEFF entry emits a per-engine
    # EVENT_SEMAPHORE_RANGE_CLEAR over this whole range (~31ns/sem) and the
    # exit resets it again; we use ~20 sems, not 106
    _orig_range = bass.get_kernel_semaphore_range
    bass.get_kernel_semaphore_range = lambda: range(150, 182)
    try:
        nc = bacc.Bacc("TRN2", target_bir_lowering=False, debug=False)
    finally:
        bass.get_kernel_semaphore_range = _orig_range

    x = nc.dram_tensor("x", [ROWS, FEAT], mybir.dt.float32, kind="ExternalInput")
    labels = nc.dram_tensor("labels", [P, NT], mybir.dt.int32, kind="ExternalInput")
    centers = nc.dram_tensor(
        "centers", [NUM_CLASSES, FEAT], mybir.dt.float32, kind="ExternalInput"
    )
    partial = nc.dram_tensor("partial", [P, NT], mybir.dt.float32, kind="ExternalOutput")

    from contextlib import ExitStack

    with ExitStack() as ctx:
        dbuf = ctx.enter_context(nc.sbuf_tensor([P, NT, FEAT], mybir.dt.float32))
        xbuf = ctx.enter_context(nc.sbuf_tensor([P, NT, FEAT], mybir.dt.float32))
        cbuf = ctx.enter_context(nc.sbuf_tensor([P, NT, FEAT], mybir.dt.float32))
        sq = ctx.enter_context(nc.sbuf_tensor([P, NT, FEAT], mybir.dt.float32))
        lab_sb = ctx.enter_context(nc.sbuf_tensor([P, NT], mybir.dt.int32))
        acc = ctx.enter_context(nc.sbuf_tensor([P, NT], mybir.dt.float32))
        warm = ctx.enter_context(nc.sbuf_tensor([P, 1, FEAT], mybir.dt.float32))
        lab_dummy = ctx.enter_context(nc.sbuf_tensor([P, 1], mybir.dt.int32))
        lsem = ctx.enter_context(nc.semaphore("lsem"))
        xsems = [ctx.enter_context(nc.semaphore(f"x{t}sem")) for t in range(NT)]
        gsems = [ctx.enter_context(nc.semaphore(f"g{t}sem")) for t in range(NT)]
        osem = ctx.enter_context(nc.semaphore("osem"))
        asem = ctx.enter_context(nc.semaphore("asem"))
        msem = ctx.enter_context(nc.semaphore("msem"))
        vsem = ctx.enter_context(nc.semaphore("vsem"))
        wsem = ctx.enter_context(nc.semaphore("wsem"))
        block = ctx.enter_context(nc.Block())

        @block.sync
        def _(sync):
            sync.dma_start(out=lab_sb[:], in_=labels.ap()).then_inc(lsem, 16)
            sync.dma_start(
                out=xbuf[:, 0, :], in_=x.ap()[0:P, :]
            ).then_inc(xsems[0], 16)
            sync.wait_ge(asem, NT)
            sync.dma_start(out=partial.ap(), in_=acc[:]).then_inc(osem, 16)
            sync.wait_ge(osem, 16)

        @block.gpsimd
        def _(g):
            # warm up the SWDGE worker before inputs land: lazy first-use
            # init (~2.4us) then overlaps the entry phase
            g.memset(lab_dummy[:], 0).then_inc(msem, 1)
            g.wait_ge(msem, 1)
            g.indirect_dma_start(
                out=warm[:16, 0, :64],
                out_offset=None,
                in_=centers.ap(),
                in_offset=bass.IndirectOffsetOnAxis(ap=lab_dummy[:16, 0:1], axis=0),
            ).then_inc(wsem, 16)
            g.wait_ge(lsem, 16)
            for t in range(NT):
                g.indirect_dma_start(
                    out=cbuf[:, t, :],
                    out_offset=None,
                    in_=centers.ap(),
                    in_offset=bass.IndirectOffsetOnAxis(
                        ap=lab_sb[:, t : t + 1], axis=0
                    ),
                ).then_inc(gsems[t], 16)

        @block.vector
        def _(v):
            for t in range(NT):
                v.wait_ge(xsems[t], 16)
                v.wait_ge(gsems[t], 16)
                v.tensor_tensor(
                    out=dbuf[:, t, :],
                    in0=xbuf[:, t, :],
                    in1=cbuf[:, t, :],
                    op=mybir.AluOpType.subtract,
                ).then_inc(vsem, 1)

        @block.scalar
        def _(s):
            for t in range(1, NT):
                s.dma_start(
                    out=xbuf[:, t, :], in_=x.ap()[t * P : (t + 1) * P, :]
                ).then_inc(xsems[t], 16)
            for t in range(NT):
                s.wait_ge(vsem, t + 1)
                s.activation(
                    out=sq[:, t, :],
                    in_=dbuf[:, t, :],
                    func=mybir.ActivationFunctionType.Square,
                    accum_out=acc[:, t : t + 1],
                ).then_inc(asem, 1)

    nc.compile()
    return nc


def _prepare_in_maps(x, centers, labels):
    x = np.asarray(x, dtype=np.float32)
    x = np.ascontiguousarray(x)
    centers = np.ascontiguousarray(np.asarray(centers, dtype=np.float32))
    lab = np.asarray(labels).astype(np.int32).reshape(N_CORES, NT, P)
    lab = np.ascontiguousarray(lab.transpose(0, 2, 1))
    xs = x.reshape(N_CORES, ROWS, FEAT)
    return [
        {"x": xs[i], "labels": lab[i], "centers": centers} for i in range(N_CORES)
    ]


def kernel(x, centers, labels):
    from concourse.bass_utils import run_bass_kernel_spmd

    if "nc" not in _CACHE:
        _CACHE["nc"] = _build_nc()
    nc = _CACHE["nc"]

    in_maps = _prepare_in_maps(x, centers, labels)
    res = run_bass_kernel_spmd(nc, in_maps, core_ids=list(range(N_CORES)))

    total = np.float64(0.0)
    for r in res.results:
        total += r["partial"].astype(np.float64).sum()
    total += np.float64(BATCH) * (NUM_CLASSES - 1) * 1e-12
    return np.float32(total / BATCH)
